# revision 7
# baseline (speedup 1.0000x reference)
"""Trainium2 Bass kernel for nn_CrossAttention (B=4, C=256, H=W=64).

Sharding: 8 cores = (batch b, branch br). Each core computes ONE branch's
full [1, N] output row for its batch:
  br=0: q,k from x1, v from x2;  br=1: q,k from x2, v from x1.
Host passes role-named inputs (xqk, xv, xcr=x1-for-combine) so the SPMD
program is branch-agnostic. This halves the k/v projection work vs
query-half sharding (no duplication across the batch pair).

Per core, for all N=4096 query rows i:
  q = Wq xqk + bq        [32, 4096] stored 4x row-replicated as q4 [128, N]
  k = Wk xqk             [32, 4096] 4x row-replicated   (bk softmax-invariant)
  vT = (Wv xv)^T         [4096, 256] bf16  (bv folded into bc_eff on host)
  S^T[j, i] = k_j . q_i  ; E = exp(S^T) bf16  (|S| <~ 30, exp safe in f32)
  r[i] = sum_j E[j, i]   via DVE bf16 accumulation (2 interleaved accums)
                         + one K=128 ones-matmul fold per block
  1/r via reciprocal_approx_fast; broadcast to 128 partitions via a K=1
  ones outer-product matmul (no DRAM round trip)
  att = (vT^T E) / r ; comb = Wc [xcr; att] + bc_eff ; out = sum_c |comb|

Pipelining: 8 blocks of 512 query cols; within a block the attended
matmuls run one j-group behind the score matmuls (PE never waits on Act
exp); each block's tail (r fold/recip/broadcast/normalize/combine) is
issued interleaved into the NEXT block's matmul stream, so the PE queue
never drains and HAM stays at 2.4 GHz. The xcr combine input is DMA'd
at phase-2 start, hidden under the first block's compute.
PSUM: 4 score staging + 3 attended accumulators + 1 rotating tail bank.
"""

import numpy as np
import ml_dtypes

import concourse.bass as bass
import concourse.bacc as bacc
import concourse.tile as tile
import concourse.mybir as mybir
from concourse.bass_utils import run_bass_kernel_spmd

B, C, HH, WW = 4, 256, 64, 64
N = HH * WW          # 4096
CQK = 32
IH = N // 2
NCORES = 8
NJC = N // 128       # 32 key-dim 128-chunks
NG = NJC // 2        # 16 groups of 2 key-chunks
NBLK = N // 512      # 8 query blocks

F32 = mybir.dt.float32
F32R = mybir.dt.float32r
BF16 = mybir.dt.bfloat16
AF = mybir.ActivationFunctionType


def build_program(nc, tc):
    # ---- DRAM I/O ------------------------------------------------------
    dram = {}
    for name, shape, dt in [
        ("xqk", [2, 128, N], F32R), ("xv", [2, 128, N], F32R),
        ("xcr", [2, 128, N], F32R),
        ("wqt", [2, 128, 128], F32R), ("wkt", [2, 128, 128], F32R),
        ("wvt", [2, 128, C], F32R),
        ("wctx", [2, 128, C], F32R), ("wcta", [2, 128, C], BF16),
        ("bq", [128, 1], F32), ("bce", [128, 2], F32),
    ]:
        dram[name] = nc.dram_tensor(name, shape, dt, kind="ExternalInput").ap()
    out_d = nc.dram_tensor("out", [1, N], F32, kind="ExternalOutput").ap()

    import contextlib
    with contextlib.ExitStack() as ctx:
        persist = ctx.enter_context(tc.tile_pool(name="persist", bufs=1))

        wq_sb = persist.tile([128, 2, 128], F32R, tag="wq")
        wk_sb = persist.tile([128, 2, 128], F32R, tag="wk")
        wv_sb = persist.tile([128, 2, C], F32R, tag="wv")
        wcx_sb = persist.tile([128, 2, C], F32R, tag="wcx")
        wca_sb = persist.tile([128, 2, C], BF16, tag="wca")
        bq_sb = persist.tile([128, 1], F32, tag="bq")
        bce_sb = persist.tile([128, 2], F32, tag="bce")
        ones_bf = persist.tile([128, 1], BF16, tag="ones")
        ones_row = persist.tile([1, 128], BF16, tag="ones_row")

        for w, t in [("wqt", wq_sb), ("wkt", wk_sb), ("wvt", wv_sb),
                     ("wctx", wcx_sb), ("wcta", wca_sb)]:
            for kc in range(2):
                nc.sync.dma_start(out=t[:, kc, :], in_=dram[w][kc])
        nc.sync.dma_start(out=bq_sb, in_=dram["bq"])
        nc.sync.dma_start(out=bce_sb, in_=dram["bce"])
        nc.vector.memset(ones_bf, 1.0)
        nc.vector.memset(ones_row, 1.0)

        q4_sb = persist.tile([128, N], F32R, tag="q4")
        k4_sb = [persist.tile([128, IH], F32R, tag=f"k{h}", name=f"k{h}")
                 for h in range(2)]
        vT_sb = [persist.tile([128, (NJC // 2) * C], BF16, tag=f"vt{h}",
                              name=f"vt{h}") for h in range(2)]
        att_sb = [persist.tile([128, N], BF16, tag=f"att{c2}",
                               name=f"att{c2}") for c2 in range(2)]
        xcr_sb = persist.tile([128, 2, N], F32R, tag="xcr")

        # ---- phase 1: projections -------------------------------------
        with tc.tile_pool(name="proj_sb", bufs=4) as proj_sb, \
             tc.tile_pool(name="ps_kq", bufs=3, space="PSUM") as ps_kq, \
             tc.tile_pool(name="ps_vt", bufs=2, space="PSUM") as ps_vt:

            xq = [proj_sb.tile([128, 2, IH], F32R, tag="xf", name=f"xq{h}")
                  for h in range(2)]
            xvt = [proj_sb.tile([128, 2, IH], F32R, tag="xf", name=f"xv{h}")
                   for h in range(2)]

            def load_half(src, dst, h):
                # 512-col chunks in consumption order so the first
                # projection matmul can start ~2.5us in, not after the
                # whole 8MB input transfer.
                for jb in range(4):
                    sl = bass.ts(jb, 512)
                    for kc in range(2):
                        nc.sync.dma_start(
                            out=dst[:, kc, sl],
                            in_=dram[src][kc][:, h * IH + jb * 512:
                                              h * IH + (jb + 1) * 512])

            load_half("xqk", xq[0], 0)
            load_half("xv", xvt[0], 0)
            load_half("xqk", xq[1], 1)
            load_half("xv", xvt[1], 1)

            def qk_proj(h):
                xap = [xq[h][:, 0, :], xq[h][:, 1, :]]
                for jb in range(4):
                    sl = bass.ts(jb, 512)
                    osl = bass.ts(h * 4 + jb, 512)
                    qp = ps_kq.tile([128, 512], F32, tag="kq", name="qp")
                    for kc in range(2):
                        nc.tensor.matmul(qp, wq_sb[:, kc, :], xap[kc][:, sl],
                                         start=(kc == 0), stop=(kc == 1))
                    nc.scalar.activation(q4_sb[:, osl], qp, AF.Identity,
                                         bias=bq_sb)
                    kp = ps_kq.tile([128, 512], F32, tag="kq", name="kp")
                    for kc in range(2):
                        nc.tensor.matmul(kp, wk_sb[:, kc, :], xap[kc][:, sl],
                                         start=(kc == 0), stop=(kc == 1))
                    nc.scalar.activation(k4_sb[h][:, sl], kp, AF.Copy)

            def v_proj(h):
                xap = [xvt[h][:, 0, :], xvt[h][:, 1, :]]
                for g in range(4):
                    vtp = ps_vt.tile([128, 4, C], F32, tag="vt", name="vtp")
                    for s in range(4):
                        jsub = g * 4 + s
                        for kc in range(2):
                            nc.tensor.matmul(
                                vtp[:, s, :],
                                xap[kc][:, bass.ts(jsub, 128)],
                                wv_sb[:, kc, :],
                                start=(kc == 0), stop=(kc == 1))
                    nc.vector.tensor_copy(
                        vT_sb[h][:, bass.ds(g * 4 * C, 4 * C)],
                        vtp.rearrange("p a c -> p (a c)"))

            qk_proj(0)
            v_proj(0)
            qk_proj(1)
            v_proj(1)

        # ---- phase 2: attention + fused combine, 1-block pipelined ----
        with tc.tile_pool(name="attn_sb", bufs=1) as attn_sb, \
             tc.tile_pool(name="ps_st", bufs=1, space="PSUM") as ps_st, \
             tc.tile_pool(name="ps_att", bufs=1, space="PSUM") as ps_att, \
             tc.tile_pool(name="ps_tail", bufs=1, space="PSUM") as ps_tail:

            # combine input: DMA'd here so it doesn't contend with phase 1;
            # first needed by block 0's tail ~26us into phase 2.
            for kc in range(2):
                nc.sync.dma_start(out=xcr_sb[:, kc, :], in_=dram["xcr"][kc])

            state = {}

            def tail_ops(p, g, pst):
                pisl = bass.ts(p, 512)
                if g == 0:
                    for c2 in range(2):
                        nc.vector.tensor_copy(att_sb[c2][:, pisl],
                                              pst["attp"][c2])
                elif g == 1:
                    rft = ps_tail.tile([1, 512], F32, tag="tail", name="rft")
                    nc.tensor.matmul(rft, ones_bf, pst["racc"][0],
                                     start=True, stop=False)
                    nc.tensor.matmul(rft, ones_bf, pst["racc"][1],
                                     start=False, stop=True)
                    pst["rft"] = rft
                elif g == 2:
                    rr_in = attn_sb.tile([1, 512], F32, tag="rr_in", bufs=2,
                                         name="rr_in")
                    nc.vector.tensor_copy(rr_in, pst["rft"])
                    rr = attn_sb.tile([1, 512], F32, tag="rr", bufs=2,
                                      name="rr")
                    nc.vector.reciprocal_approx_fast(out=rr, in_=rr_in)
                    rr_bf = attn_sb.tile([1, 512], BF16, tag="rr_bf", bufs=2,
                                         name="rr_bf")
                    nc.vector.tensor_copy(rr_bf, rr)
                    pst["rr_bf"] = rr_bf
                elif g == 5:
                    rrb = ps_tail.tile([128, 512], F32, tag="tail",
                                       name="rrb")
                    nc.tensor.matmul(rrb, ones_row, pst["rr_bf"],
                                     start=True, stop=True)
                    pst["rrb"] = rrb
                elif g == 6:
                    for c2 in range(2):
                        a = att_sb[c2][:, pisl]
                        nc.vector.tensor_mul(a, a, pst["rrb"])
                elif g == 8 or g == 10:
                    c2 = 0 if g == 8 else 1
                    cp = ps_tail.tile([128, 512], F32, tag="tail",
                                      name=f"cp{c2}")
                    for kc in range(2):
                        nc.tensor.matmul(cp, wcx_sb[:, kc, bass.ts(c2, 128)],
                                         xcr_sb[:, kc, pisl],
                                         start=(kc == 0), stop=False)
                    for kc in range(2):
                        nc.tensor.matmul(cp, wca_sb[:, kc, bass.ts(c2, 128)],
                                         att_sb[kc][:, pisl],
                                         start=False, stop=(kc == 1))
                    absb = attn_sb.tile([128, 512], BF16, tag="absb",
                                        bufs=4, name=f"absb{c2}")
                    nc.scalar.activation(absb, cp, AF.Abs,
                                         bias=bce_sb[:, c2:c2 + 1])
                    pst[f"absb{c2}"] = absb
                elif g == 12:
                    outp = ps_tail.tile([1, 512], F32, tag="tail",
                                        name="outp")
                    nc.tensor.matmul(outp, ones_bf, pst["absb0"],
                                     start=True, stop=False)
                    nc.tensor.matmul(outp, ones_bf, pst["absb1"],
                                     start=False, stop=True)
                    pst["outp"] = outp
                elif g == 13:
                    osb = attn_sb.tile([1, 512], F32, tag="osb", bufs=2,
                                       name="osb")
                    nc.vector.tensor_copy(osb, pst["outp"])
                    nc.sync.dma_start(out=out_d[0:1, pisl], in_=osb)

            def attended(st, g, first, last):
                jh = g // (NG // 2)
                for t in range(2):
                    jloc = (2 * g + t) - jh * (NJC // 2)
                    for c2 in range(2):
                        nc.tensor.matmul(
                            st["attp"][c2],
                            vT_sb[jh][:, bass.ds(jloc * C + c2 * 128, 128)],
                            st["est"][g % 8][:, t, :],
                            start=(first and t == 0),
                            stop=(last and t == 1))

            for mi in range(NBLK + 1):
                cur = mi if mi < NBLK else None
                if cur is not None:
                    isl = bass.ts(cur, 512)
                    st = {
                        "attp": [ps_att.tile([128, 512], F32, tag="attp",
                                             bufs=3, name=f"attp{c2}")
                                 for c2 in range(2)],
                        "racc": [attn_sb.tile([128, 512], BF16, tag="racc",
                                              bufs=4, name=f"racc{t}")
                                 for t in range(2)],
                        "est": [None] * 8,
                    }
                    state[mi] = st
                for g in range(16 if cur is not None else 14):
                    if cur is not None:
                        jh = g // (NG // 2)
                        jloc = [(2 * g + t) - jh * (NJC // 2)
                                for t in range(2)]
                        stp = ps_st.tile([128, 2, 512], F32, tag="stp",
                                         bufs=2, name="stp")
                        for t in range(2):
                            nc.tensor.matmul(
                                stp[:, t, :],
                                k4_sb[jh][32 * t:32 * (t + 1),
                                          bass.ts(jloc[t], 128)],
                                q4_sb[32 * t:32 * (t + 1), isl],
                                start=True, stop=True,
                                tile_position=(32 * t, 0))
                        est = attn_sb.tile([128, 2, 512], BF16,
                                           tag="est", bufs=8, name="est")
                        st["est"][g % 8] = est
                        nc.scalar.activation(
                            est.rearrange("p a n -> p (a n)"),
                            stp.rearrange("p a n -> p (a n)"), AF.Exp)
                        # attended runs one group behind the scores
                        if g > 0:
                            attended(st, g - 1, first=(g == 1), last=False)
                        # DVE r-accumulation (bf16, 2 interleaved accums)
                        for t in range(2):
                            if g == 0:
                                nc.vector.tensor_copy(st["racc"][t],
                                                      est[:, t, :])
                            else:
                                nc.vector.tensor_add(st["racc"][t],
                                                     st["racc"][t],
                                                     est[:, t, :])
                    if mi > 0:
                        tail_ops(mi - 1, g, state[mi - 1])
                if cur is not None:
                    attended(st, 15, first=False, last=True)


_NC_CACHE = {}


def _get_nc():
    if "nc" not in _NC_CACHE:
        nc = bacc.Bacc("TRN2", debug=False, enable_asserts=False,
                       target_bir_lowering=False, enable_partition_id=False)
        with tile.TileContext(nc) as tc:
            build_program(nc, tc)
        nc.compile()
        _NC_CACHE["nc"] = nc
    return _NC_CACHE["nc"]


def host_inputs(x1, x2, Wq, bq, Wk, bk, Wv, bv, Wc, bc):
    """Build the 8 per-core input maps (host-side sharding/layout only)."""
    f = np.float32
    x1 = np.asarray(x1, f); x2 = np.asarray(x2, f)
    Wq = np.asarray(Wq, f); bq = np.asarray(bq, f)
    Wk = np.asarray(Wk, f)
    Wv = np.asarray(Wv, f); bv = np.asarray(bv, f)
    Wc = np.asarray(Wc, f); bc = np.asarray(bc, f)

    # 4x row-replicated q/k projection weights -> q4/k4 [128, n] layouts
    Wq4 = np.tile(Wq, (4, 1))            # [128, 256]
    Wk4 = np.tile(Wk, (4, 1))
    wqt = np.ascontiguousarray(Wq4.T.reshape(2, 128, 128))
    wkt = np.ascontiguousarray(Wk4.T.reshape(2, 128, 128))
    bq4 = np.tile(bq, 4).reshape(128, 1).copy()
    wvt = np.ascontiguousarray(Wv.T.reshape(2, 128, C))
    WcT = np.ascontiguousarray(Wc.T)     # [512, 256]
    wctx = WcT[:C].reshape(2, 128, C).copy()
    wcta = WcT[C:].reshape(2, 128, C).astype(ml_dtypes.bfloat16)
    bce = (bc + Wc[:, C:] @ bv).reshape(2, 128).T.copy()   # [128, 2]

    in_maps = []
    for core in range(NCORES):
        b, br = divmod(core, 2)
        x1f = np.ascontiguousarray(x1[b].reshape(C, N).reshape(2, 128, N))
        x2f = np.ascontiguousarray(x2[b].reshape(C, N).reshape(2, 128, N))
        in_maps.append({
            "xqk": x1f if br == 0 else x2f,
            "xv": x2f if br == 0 else x1f,
            "xcr": x1f,
            "wqt": wqt, "wkt": wkt, "wvt": wvt,
            "wctx": wctx, "wcta": wcta,
            "bq": bq4, "bce": bce,
        })
    return in_maps


def assemble(results):
    """results: list of 8 dicts with 'out' [1, N] -> (out1, out2) full."""
    outs = []
    for br in range(2):
        full = np.empty((B, 1, HH, WW), np.float32)
        for b in range(B):
            full[b, 0] = results[2 * b + br]["out"][0].reshape(HH, WW)
        outs.append(full)
    return outs[0], outs[1]


def kernel(x1, x2, Wq, bq, Wk, bk, Wv, bv, Wc, bc):
    in_maps = host_inputs(x1, x2, Wq, bq, Wk, bk, Wv, bv, Wc, bc)
    nc = _get_nc()
    res = run_bass_kernel_spmd(nc, in_maps, core_ids=list(range(NCORES)))
    return assemble(res.results)


# revision 14
# speedup vs baseline: 1.0218x; 1.0218x over previous
"""Trainium2 Bass kernel for nn_CrossAttention (B=4, C=256, H=W=64).

Sharding: 8 cores = (batch b, branch br). Each core computes ONE branch's
full [1, N] output row for its batch:
  br=0: q,k from x1, v from x2;  br=1: q,k from x2, v from x1.
Host passes role-named inputs (xqk, xv, xcr=x1-for-combine) so the SPMD
program is branch-agnostic. This halves the k/v projection work vs
query-half sharding (no duplication across the batch pair).

Per core, for all N=4096 query rows i:
  q = Wq xqk + bq        [32, 4096] stored 4x row-replicated as q4 [128, N]
  k = Wk xqk             [32, 4096] 4x row-replicated   (bk softmax-invariant)
  vT = (Wv xv)^T         [4096, 256] bf16  (bv folded into bc_eff on host)
  S^T[j, i] = k_j . q_i  ; E = exp(S^T) bf16  (|S| <~ 30, exp safe in f32)
  r[i] = sum_j E[j, i]   via DVE bf16 accumulation (2 interleaved accums)
                         + one K=128 ones-matmul fold per block
  1/r via reciprocal_approx_fast; broadcast to 128 partitions via a K=1
  ones outer-product matmul (no DRAM round trip)
  att = (vT^T E) / r ; comb = Wc [xcr; att] + bc_eff ; out = sum_c |comb|

Pipelining: 8 blocks of 512 query cols; within a block the attended
matmuls run one j-group behind the score matmuls (PE never waits on Act
exp); each block's tail (r fold/recip/broadcast/normalize/combine) is
issued interleaved into the NEXT block's matmul stream, so the PE queue
never drains and HAM stays at 2.4 GHz. The xcr combine input is DMA'd
at phase-2 start, hidden under the first block's compute.
PSUM: 4 score staging + 3 attended accumulators + 1 rotating tail bank.
"""

import numpy as np
import ml_dtypes

import concourse.bass as bass
import concourse.bacc as bacc
import concourse.tile as tile
import concourse.mybir as mybir
from concourse.bass_utils import run_bass_kernel_spmd

B, C, HH, WW = 4, 256, 64, 64
N = HH * WW          # 4096
CQK = 32
IH = N // 2
NCORES = 8
NJC = N // 128       # 32 key-dim 128-chunks
NG = NJC // 2        # 16 groups of 2 key-chunks
NBLK = N // 512      # 8 query blocks

F32 = mybir.dt.float32
F32R = mybir.dt.float32r
BF16 = mybir.dt.bfloat16
AF = mybir.ActivationFunctionType


def build_program(nc, tc):
    # ---- DRAM I/O ------------------------------------------------------
    dram = {}
    for name, shape, dt in [
        ("xqk", [2, 128, N], F32R), ("xv", [2, 128, N], BF16),
        ("xcr", [2, 128, N], BF16),
        ("wqt", [2, 128, 128], F32R), ("wkt", [2, 128, 128], F32R),
        ("wvt", [2, 128, C], BF16),
        ("wctx", [2, 128, C], BF16), ("wcta", [2, 128, C], BF16),
        ("bq", [128, 1], F32), ("bce", [128, 2], F32),
    ]:
        dram[name] = nc.dram_tensor(name, shape, dt, kind="ExternalInput").ap()
    out_d = nc.dram_tensor("out", [1, N], F32, kind="ExternalOutput").ap()

    import contextlib
    with contextlib.ExitStack() as ctx:
        persist = ctx.enter_context(tc.tile_pool(name="persist", bufs=1))

        wq_sb = persist.tile([128, 2, 128], F32R, tag="wq")
        wk_sb = persist.tile([128, 2, 128], F32R, tag="wk")
        wv_sb = persist.tile([128, 2, C], BF16, tag="wv")
        wcx_sb = persist.tile([128, 2, C], BF16, tag="wcx")
        wca_sb = persist.tile([128, 2, C], BF16, tag="wca")
        bq_sb = persist.tile([128, 1], F32, tag="bq")
        bce_sb = persist.tile([128, 2], F32, tag="bce")
        ones_bf = persist.tile([128, 1], BF16, tag="ones")
        ones_row = persist.tile([1, 128], BF16, tag="ones_row")

        for w, t in [("wqt", wq_sb), ("wkt", wk_sb), ("wvt", wv_sb),
                     ("wctx", wcx_sb), ("wcta", wca_sb)]:
            for kc in range(2):
                nc.sync.dma_start(out=t[:, kc, :], in_=dram[w][kc])
        nc.sync.dma_start(out=bq_sb, in_=dram["bq"])
        nc.sync.dma_start(out=bce_sb, in_=dram["bce"])
        nc.vector.memset(ones_bf, 1.0)
        nc.vector.memset(ones_row, 1.0)

        q4_sb = persist.tile([128, N], F32R, tag="q4")
        k4_sb = [persist.tile([128, IH], F32R, tag=f"k{h}", name=f"k{h}")
                 for h in range(2)]
        vT_sb = [persist.tile([128, (NJC // 2) * C], BF16, tag=f"vt{h}",
                              name=f"vt{h}") for h in range(2)]
        att_sb = [persist.tile([128, N], BF16, tag=f"att{c2}",
                               name=f"att{c2}") for c2 in range(2)]
        xcr_sb = persist.tile([128, 2, N], BF16, tag="xcr")

        # ---- phase 1: projections -------------------------------------
        with tc.tile_pool(name="proj_sb", bufs=4) as proj_sb, \
             tc.tile_pool(name="ps_kq", bufs=3, space="PSUM") as ps_kq, \
             tc.tile_pool(name="ps_vt", bufs=2, space="PSUM") as ps_vt:

            xq = [proj_sb.tile([128, 2, IH], F32R, tag="xq", name=f"xq{h}")
                  for h in range(2)]
            xvt = [proj_sb.tile([128, 2, IH], BF16, tag="xv", name=f"xv{h}")
                   for h in range(2)]

            def load_half(src, dst, h):
                # 1024-col chunks in consumption order: dma_start issue
                # costs ~630ns of sequencer time, so chunks balance issue
                # rate against letting the first projection start early.
                for jb in range(2):
                    sl = bass.ds(jb * 1024, 1024)
                    for kc in range(2):
                        nc.sync.dma_start(
                            out=dst[:, kc, sl],
                            in_=dram[src][kc][:, h * IH + jb * 1024:
                                              h * IH + (jb + 1) * 1024])

            load_half("xqk", xq[0], 0)
            load_half("xv", xvt[0], 0)
            load_half("xqk", xq[1], 1)
            load_half("xv", xvt[1], 1)

            def qk_proj(h):
                xap = [xq[h][:, 0, :], xq[h][:, 1, :]]
                for jb in range(4):
                    sl = bass.ts(jb, 512)
                    osl = bass.ts(h * 4 + jb, 512)
                    qp = ps_kq.tile([128, 512], F32, tag="kq", name="qp")
                    for kc in range(2):
                        nc.tensor.matmul(qp, wq_sb[:, kc, :], xap[kc][:, sl],
                                         start=(kc == 0), stop=(kc == 1))
                    nc.scalar.activation(q4_sb[:, osl], qp, AF.Identity,
                                         bias=bq_sb)
                    kp = ps_kq.tile([128, 512], F32, tag="kq", name="kp")
                    for kc in range(2):
                        nc.tensor.matmul(kp, wk_sb[:, kc, :], xap[kc][:, sl],
                                         start=(kc == 0), stop=(kc == 1))
                    nc.scalar.activation(k4_sb[h][:, sl], kp, AF.Copy)

            def v_proj(h):
                xap = [xvt[h][:, 0, :], xvt[h][:, 1, :]]
                for g in range(4):
                    vtp = ps_vt.tile([128, 4, C], F32, tag="vt", name="vtp")
                    for s in range(4):
                        jsub = g * 4 + s
                        for kc in range(2):
                            nc.tensor.matmul(
                                vtp[:, s, :],
                                xap[kc][:, bass.ts(jsub, 128)],
                                wv_sb[:, kc, :],
                                start=(kc == 0), stop=(kc == 1))
                    nc.vector.tensor_copy(
                        vT_sb[h][:, bass.ds(g * 4 * C, 4 * C)],
                        vtp.rearrange("p a c -> p (a c)"))

            qk_proj(0)
            v_proj(0)
            qk_proj(1)
            v_proj(1)

        # ---- phase 2: attention + fused combine, 1-block pipelined ----
        with tc.tile_pool(name="attn_sb", bufs=1) as attn_sb, \
             tc.tile_pool(name="ps_st", bufs=1, space="PSUM") as ps_st, \
             tc.tile_pool(name="ps_att", bufs=1, space="PSUM") as ps_att, \
             tc.tile_pool(name="ps_tail", bufs=1, space="PSUM") as ps_tail:

            # combine input: DMA'd here so it doesn't contend with phase 1;
            # first needed by block 0's tail ~26us into phase 2.
            for kc in range(2):
                nc.sync.dma_start(out=xcr_sb[:, kc, :], in_=dram["xcr"][kc])

            state = {}

            def tail_ops(p, g, pst):
                pisl = bass.ts(p, 512)
                if g == 0:
                    for c2 in range(2):
                        nc.vector.tensor_copy(att_sb[c2][:, pisl],
                                              pst["attp"][c2])
                elif g == 1:
                    rft = ps_tail.tile([1, 512], F32, tag="tail", name="rft")
                    nc.tensor.matmul(rft, ones_bf, pst["racc"][0],
                                     start=True, stop=False)
                    nc.tensor.matmul(rft, ones_bf, pst["racc"][1],
                                     start=False, stop=True)
                    pst["rft"] = rft
                elif g == 2:
                    rr = attn_sb.tile([1, 512], F32, tag="rr", bufs=2,
                                      name="rr")
                    nc.vector.reciprocal_approx_fast(out=rr, in_=pst["rft"])
                    rr_bf = attn_sb.tile([1, 512], BF16, tag="rr_bf", bufs=2,
                                         name="rr_bf")
                    nc.vector.tensor_copy(rr_bf, rr)
                    pst["rr_bf"] = rr_bf
                elif g == 5:
                    rrb = ps_tail.tile([128, 512], F32, tag="tail",
                                       name="rrb")
                    nc.tensor.matmul(rrb, ones_row, pst["rr_bf"],
                                     start=True, stop=True)
                    pst["rrb"] = rrb
                elif g == 6:
                    for c2 in range(2):
                        a = att_sb[c2][:, pisl]
                        nc.vector.tensor_mul(a, a, pst["rrb"])
                elif g == 8 or g == 10:
                    c2 = 0 if g == 8 else 1
                    cp = ps_tail.tile([128, 512], F32, tag="tail",
                                      name=f"cp{c2}")
                    for kc in range(2):
                        nc.tensor.matmul(cp, wcx_sb[:, kc, bass.ts(c2, 128)],
                                         xcr_sb[:, kc, pisl],
                                         start=(kc == 0), stop=False)
                    for kc in range(2):
                        nc.tensor.matmul(cp, wca_sb[:, kc, bass.ts(c2, 128)],
                                         att_sb[kc][:, pisl],
                                         start=False, stop=(kc == 1))
                    absb = attn_sb.tile([128, 512], BF16, tag="absb",
                                        bufs=4, name=f"absb{c2}")
                    nc.scalar.activation(absb, cp, AF.Abs,
                                         bias=bce_sb[:, c2:c2 + 1])
                    pst[f"absb{c2}"] = absb
                elif g == 12:
                    outp = ps_tail.tile([1, 512], F32, tag="tail",
                                        name="outp")
                    nc.tensor.matmul(outp, ones_bf, pst["absb0"],
                                     start=True, stop=False)
                    nc.tensor.matmul(outp, ones_bf, pst["absb1"],
                                     start=False, stop=True)
                    pst["outp"] = outp
                elif g == 13:
                    osb = attn_sb.tile([1, 512], F32, tag="osb", bufs=2,
                                       name="osb")
                    nc.vector.tensor_copy(osb, pst["outp"])
                    nc.sync.dma_start(out=out_d[0:1, pisl], in_=osb)

            def attended(st, g, first, last):
                jh = g // (NG // 2)
                for t in range(2):
                    jloc = (2 * g + t) - jh * (NJC // 2)
                    for c2 in range(2):
                        nc.tensor.matmul(
                            st["attp"][c2],
                            vT_sb[jh][:, bass.ds(jloc * C + c2 * 128, 128)],
                            st["est"][g % 8][:, t, :],
                            start=(first and t == 0),
                            stop=(last and t == 1))

            for mi in range(NBLK + 1):
                cur = mi if mi < NBLK else None
                if cur is not None:
                    isl = bass.ts(cur, 512)
                    st = {
                        "attp": [ps_att.tile([128, 512], F32, tag="attp",
                                             bufs=3, name=f"attp{c2}")
                                 for c2 in range(2)],
                        "racc": [attn_sb.tile([128, 512], BF16, tag="racc",
                                              bufs=4, name=f"racc{t}")
                                 for t in range(2)],
                        "est": [None] * 8,
                    }
                    state[mi] = st
                for g in range(16 if cur is not None else 14):
                    if cur is not None:
                        jh = g // (NG // 2)
                        jloc = [(2 * g + t) - jh * (NJC // 2)
                                for t in range(2)]
                        stp = ps_st.tile([128, 2, 512], F32, tag="stp",
                                         bufs=2, name="stp")
                        for t in range(2):
                            nc.tensor.matmul(
                                stp[:, t, :],
                                k4_sb[jh][32 * t:32 * (t + 1),
                                          bass.ts(jloc[t], 128)],
                                q4_sb[32 * t:32 * (t + 1), isl],
                                start=True, stop=True,
                                tile_position=(32 * t, 0))
                        est = attn_sb.tile([128, 2, 512], BF16,
                                           tag="est", bufs=8, name="est")
                        st["est"][g % 8] = est
                        nc.scalar.activation(
                            est.rearrange("p a n -> p (a n)"),
                            stp.rearrange("p a n -> p (a n)"), AF.Exp)
                        # attended runs one group behind the scores
                        if g > 0:
                            attended(st, g - 1, first=(g == 1), last=False)
                        # DVE r-accumulation (bf16, 2 interleaved accums)
                        for t in range(2):
                            if g == 0:
                                nc.vector.tensor_copy(st["racc"][t],
                                                      est[:, t, :])
                            else:
                                nc.vector.tensor_add(st["racc"][t],
                                                     st["racc"][t],
                                                     est[:, t, :])
                    if mi > 0:
                        tail_ops(mi - 1, g, state[mi - 1])
                if cur is not None:
                    attended(st, 15, first=False, last=True)


_NC_CACHE = {}


def _get_nc():
    if "nc" not in _NC_CACHE:
        nc = bacc.Bacc("TRN2", debug=False, enable_asserts=False,
                       target_bir_lowering=False, enable_partition_id=False)
        with tile.TileContext(nc) as tc:
            build_program(nc, tc)
        nc.compile()
        _NC_CACHE["nc"] = nc
    return _NC_CACHE["nc"]


def host_inputs(x1, x2, Wq, bq, Wk, bk, Wv, bv, Wc, bc):
    """Build the 8 per-core input maps (host-side sharding/layout only)."""
    f = np.float32
    x1 = np.asarray(x1, f); x2 = np.asarray(x2, f)
    Wq = np.asarray(Wq, f); bq = np.asarray(bq, f)
    Wk = np.asarray(Wk, f)
    Wv = np.asarray(Wv, f); bv = np.asarray(bv, f)
    Wc = np.asarray(Wc, f); bc = np.asarray(bc, f)

    # 4x row-replicated q/k projection weights -> q4/k4 [128, n] layouts
    Wq4 = np.tile(Wq, (4, 1))            # [128, 256]
    Wk4 = np.tile(Wk, (4, 1))
    wqt = np.ascontiguousarray(Wq4.T.reshape(2, 128, 128))
    wkt = np.ascontiguousarray(Wk4.T.reshape(2, 128, 128))
    bq4 = np.tile(bq, 4).reshape(128, 1).copy()
    wvt = Wv.T.reshape(2, 128, C).astype(ml_dtypes.bfloat16)
    WcT = np.ascontiguousarray(Wc.T)     # [512, 256]
    wctx = WcT[:C].reshape(2, 128, C).astype(ml_dtypes.bfloat16)
    wcta = WcT[C:].reshape(2, 128, C).astype(ml_dtypes.bfloat16)
    bce = (bc + Wc[:, C:] @ bv).reshape(2, 128).T.copy()   # [128, 2]

    in_maps = []
    bf = ml_dtypes.bfloat16
    for core in range(NCORES):
        b, br = divmod(core, 2)
        x1f = np.ascontiguousarray(x1[b].reshape(C, N).reshape(2, 128, N))
        x2f = np.ascontiguousarray(x2[b].reshape(C, N).reshape(2, 128, N))
        in_maps.append({
            "xqk": x1f if br == 0 else x2f,
            "xv": (x2f if br == 0 else x1f).astype(bf),
            "xcr": x1f.astype(bf),
            "wqt": wqt, "wkt": wkt, "wvt": wvt,
            "wctx": wctx, "wcta": wcta,
            "bq": bq4, "bce": bce,
        })
    return in_maps


def assemble(results):
    """results: list of 8 dicts with 'out' [1, N] -> (out1, out2) full."""
    outs = []
    for br in range(2):
        full = np.empty((B, 1, HH, WW), np.float32)
        for b in range(B):
            full[b, 0] = results[2 * b + br]["out"][0].reshape(HH, WW)
        outs.append(full)
    return outs[0], outs[1]


def kernel(x1, x2, Wq, bq, Wk, bk, Wv, bv, Wc, bc):
    in_maps = host_inputs(x1, x2, Wq, bq, Wk, bk, Wv, bv, Wc, bc)
    nc = _get_nc()
    res = run_bass_kernel_spmd(nc, in_maps, core_ids=list(range(NCORES)))
    return assemble(res.results)


# revision 17
# speedup vs baseline: 1.0276x; 1.0057x over previous
"""Trainium2 Bass kernel for nn_CrossAttention (B=4, C=256, H=W=64).

Sharding: 8 cores = (batch b, branch br). Each core computes ONE branch's
full [1, N] output row for its batch:
  br=0: q,k from x1, v from x2;  br=1: q,k from x2, v from x1.
Host passes role-named inputs (xqk, xv, xcr=x1-for-combine) so the SPMD
program is branch-agnostic. This halves the k/v projection work vs
query-half sharding (no duplication across the batch pair).

Per core, for all N=4096 query rows i:
  q = Wq xqk + bq        [32, 4096] stored 4x row-replicated as q4 [128, N]
  k = Wk xqk             [32, 4096] 4x row-replicated   (bk softmax-invariant)
  vT = (Wv xv)^T         [4096, 256] bf16  (bv folded into bc_eff on host)
  S^T[j, i] = k_j . q_i  ; E = exp(S^T) bf16  (|S| <~ 30, exp safe in f32)
  r[i] = sum_j E[j, i]   via DVE bf16 accumulation (2 interleaved accums)
                         + one K=128 ones-matmul fold per block
  1/r via reciprocal_approx_fast; broadcast to 128 partitions via a K=1
  ones outer-product matmul (no DRAM round trip)
  att = (vT^T E) / r ; comb = Wc [xcr; att] + bc_eff ; out = sum_c |comb|

Pipelining: 8 blocks of 512 query cols; within a block the attended
matmuls run one j-group behind the score matmuls (PE never waits on Act
exp); each block's tail (r fold/recip/broadcast/normalize/combine) is
issued interleaved into the NEXT block's matmul stream, so the PE queue
never drains and HAM stays at 2.4 GHz. The xcr combine input is DMA'd
at phase-2 start, hidden under the first block's compute.
PSUM: 4 score staging + 3 attended accumulators + 1 rotating tail bank.
"""

import numpy as np
import ml_dtypes

import concourse.bass as bass
import concourse.bacc as bacc
import concourse.tile as tile
import concourse.mybir as mybir
from concourse.bass_utils import run_bass_kernel_spmd

B, C, HH, WW = 4, 256, 64, 64
N = HH * WW          # 4096
CQK = 32
IH = N // 2
NCORES = 8
NJC = N // 128       # 32 key-dim 128-chunks
NG = NJC // 2        # 16 groups of 2 key-chunks
NBLK = N // 512      # 8 query blocks

F32 = mybir.dt.float32
F32R = mybir.dt.float32r
BF16 = mybir.dt.bfloat16
AF = mybir.ActivationFunctionType


def build_program(nc, tc):
    # ---- DRAM I/O ------------------------------------------------------
    dram = {}
    for name, shape, dt in [
        ("xqk", [2, 128, N], F32R), ("xv", [2, 128, N], BF16),
        ("xcr", [2, 128, N], BF16),
        ("wqt", [2, 128, 128], F32R), ("wkt", [2, 128, 128], F32R),
        ("wvt", [2, 128, C], BF16),
        ("wctx", [2, 128, C], BF16), ("wcta", [2, 128, C], BF16),
        ("bq", [128, 1], F32), ("bce", [128, 2], F32),
    ]:
        dram[name] = nc.dram_tensor(name, shape, dt, kind="ExternalInput").ap()
    out_d = nc.dram_tensor("out", [1, N], F32, kind="ExternalOutput").ap()

    import contextlib
    with contextlib.ExitStack() as ctx:
        persist = ctx.enter_context(tc.tile_pool(name="persist", bufs=1))

        wq_sb = persist.tile([128, 2, 128], F32R, tag="wq")
        wk_sb = persist.tile([128, 2, 128], F32R, tag="wk")
        wv_sb = persist.tile([128, 2, C], BF16, tag="wv")
        wcx_sb = persist.tile([128, 2, C], BF16, tag="wcx")
        wca_sb = persist.tile([128, 2, C], BF16, tag="wca")
        bq_sb = persist.tile([128, 1], F32, tag="bq")
        bce_sb = persist.tile([128, 2], F32, tag="bce")
        ones_bf = persist.tile([128, 1], BF16, tag="ones")
        ones_row = persist.tile([1, 128], BF16, tag="ones_row")

        # weight/bias loads issued from the (otherwise idle) scalar queue:
        # each dma_start costs ~630ns of sequencer issue time, and these
        # must not delay the x loads on the sync queue.
        for w, t in [("wqt", wq_sb), ("wkt", wk_sb), ("wvt", wv_sb),
                     ("wctx", wcx_sb), ("wcta", wca_sb)]:
            for kc in range(2):
                nc.scalar.dma_start(out=t[:, kc, :], in_=dram[w][kc])
        nc.scalar.dma_start(out=bq_sb, in_=dram["bq"])
        nc.scalar.dma_start(out=bce_sb, in_=dram["bce"])
        nc.vector.memset(ones_bf, 1.0)
        nc.vector.memset(ones_row, 1.0)

        q4_sb = persist.tile([128, N], F32R, tag="q4")
        k4_sb = [persist.tile([128, IH], F32R, tag=f"k{h}", name=f"k{h}")
                 for h in range(2)]
        vT_sb = [persist.tile([128, (NJC // 2) * C], BF16, tag=f"vt{h}",
                              name=f"vt{h}") for h in range(2)]
        att_sb = [persist.tile([128, N], BF16, tag=f"att{c2}",
                               name=f"att{c2}") for c2 in range(2)]
        xcr_sb = persist.tile([128, 2, N], BF16, tag="xcr")

        # ---- phase 1: projections -------------------------------------
        with tc.tile_pool(name="proj_sb", bufs=4) as proj_sb, \
             tc.tile_pool(name="ps_kq", bufs=3, space="PSUM") as ps_kq, \
             tc.tile_pool(name="ps_vt", bufs=2, space="PSUM") as ps_vt:

            xq = [proj_sb.tile([128, 2, IH], F32R, tag="xq", name=f"xq{h}")
                  for h in range(2)]
            xvt = [proj_sb.tile([128, 2, IH], BF16, tag="xv", name=f"xv{h}")
                   for h in range(2)]

            def load_half(src, dst, h, eng):
                # 1024-col chunks in consumption order: dma_start issue
                # costs ~630ns of sequencer time, so chunks balance issue
                # rate against letting the first projection start early.
                # xqk goes via sync, xv via the idle vector queue so both
                # streams issue in parallel.
                for jb in range(2):
                    sl = bass.ds(jb * 1024, 1024)
                    for kc in range(2):
                        eng.dma_start(
                            out=dst[:, kc, sl],
                            in_=dram[src][kc][:, h * IH + jb * 1024:
                                              h * IH + (jb + 1) * 1024])

            load_half("xqk", xq[0], 0, nc.sync)
            load_half("xv", xvt[0], 0, nc.gpsimd)
            load_half("xqk", xq[1], 1, nc.sync)
            load_half("xv", xvt[1], 1, nc.gpsimd)

            def qk_proj(h):
                xap = [xq[h][:, 0, :], xq[h][:, 1, :]]
                for jb in range(4):
                    sl = bass.ts(jb, 512)
                    osl = bass.ts(h * 4 + jb, 512)
                    qp = ps_kq.tile([128, 512], F32, tag="kq", name="qp")
                    for kc in range(2):
                        nc.tensor.matmul(qp, wq_sb[:, kc, :], xap[kc][:, sl],
                                         start=(kc == 0), stop=(kc == 1))
                    nc.scalar.activation(q4_sb[:, osl], qp, AF.Identity,
                                         bias=bq_sb)
                    kp = ps_kq.tile([128, 512], F32, tag="kq", name="kp")
                    for kc in range(2):
                        nc.tensor.matmul(kp, wk_sb[:, kc, :], xap[kc][:, sl],
                                         start=(kc == 0), stop=(kc == 1))
                    nc.scalar.activation(k4_sb[h][:, sl], kp, AF.Copy)

            def v_proj(h):
                xap = [xvt[h][:, 0, :], xvt[h][:, 1, :]]
                for g in range(4):
                    vtp = ps_vt.tile([128, 4, C], F32, tag="vt", name="vtp")
                    for s in range(4):
                        jsub = g * 4 + s
                        for kc in range(2):
                            nc.tensor.matmul(
                                vtp[:, s, :],
                                xap[kc][:, bass.ts(jsub, 128)],
                                wv_sb[:, kc, :],
                                start=(kc == 0), stop=(kc == 1))
                    nc.vector.tensor_copy(
                        vT_sb[h][:, bass.ds(g * 4 * C, 4 * C)],
                        vtp.rearrange("p a c -> p (a c)"))

            qk_proj(0)
            v_proj(0)
            qk_proj(1)
            v_proj(1)

        # ---- phase 2: attention + fused combine, 1-block pipelined ----
        with tc.tile_pool(name="attn_sb", bufs=1) as attn_sb, \
             tc.tile_pool(name="ps_st", bufs=1, space="PSUM") as ps_st, \
             tc.tile_pool(name="ps_att", bufs=1, space="PSUM") as ps_att, \
             tc.tile_pool(name="ps_tail", bufs=1, space="PSUM") as ps_tail:

            # combine input: DMA'd here so it doesn't contend with phase 1;
            # first needed by block 0's tail ~26us into phase 2.
            for kc in range(2):
                nc.sync.dma_start(out=xcr_sb[:, kc, :], in_=dram["xcr"][kc])

            state = {}

            def tail_ops(p, g, pst):
                pisl = bass.ts(p, 512)
                if g == 0:
                    for c2 in range(2):
                        nc.vector.tensor_copy(att_sb[c2][:, pisl],
                                              pst["attp"][c2])
                elif g == 1:
                    rft = ps_tail.tile([1, 512], F32, tag="tail", name="rft")
                    nc.tensor.matmul(rft, ones_bf, pst["racc"][0],
                                     start=True, stop=False)
                    nc.tensor.matmul(rft, ones_bf, pst["racc"][1],
                                     start=False, stop=True)
                    pst["rft"] = rft
                elif g == 2:
                    rr = attn_sb.tile([1, 512], F32, tag="rr", bufs=2,
                                      name="rr")
                    nc.vector.reciprocal_approx_fast(out=rr, in_=pst["rft"])
                    rr_bf = attn_sb.tile([1, 512], BF16, tag="rr_bf", bufs=2,
                                         name="rr_bf")
                    nc.vector.tensor_copy(rr_bf, rr)
                    pst["rr_bf"] = rr_bf
                elif g == 5:
                    rrb = ps_tail.tile([128, 512], F32, tag="tail",
                                       name="rrb")
                    nc.tensor.matmul(rrb, ones_row, pst["rr_bf"],
                                     start=True, stop=True)
                    pst["rrb"] = rrb
                elif g == 6:
                    for c2 in range(2):
                        a = att_sb[c2][:, pisl]
                        nc.vector.tensor_mul(a, a, pst["rrb"])
                elif g == 8 or g == 10:
                    c2 = 0 if g == 8 else 1
                    cp = ps_tail.tile([128, 512], F32, tag="tail",
                                      name=f"cp{c2}")
                    for kc in range(2):
                        nc.tensor.matmul(cp, wcx_sb[:, kc, bass.ts(c2, 128)],
                                         xcr_sb[:, kc, pisl],
                                         start=(kc == 0), stop=False)
                    for kc in range(2):
                        nc.tensor.matmul(cp, wca_sb[:, kc, bass.ts(c2, 128)],
                                         att_sb[kc][:, pisl],
                                         start=False, stop=(kc == 1))
                    absb = attn_sb.tile([128, 512], BF16, tag="absb",
                                        bufs=4, name=f"absb{c2}")
                    nc.scalar.activation(absb, cp, AF.Abs,
                                         bias=bce_sb[:, c2:c2 + 1])
                    pst[f"absb{c2}"] = absb
                elif g == 12:
                    outp = ps_tail.tile([1, 512], F32, tag="tail",
                                        name="outp")
                    nc.tensor.matmul(outp, ones_bf, pst["absb0"],
                                     start=True, stop=False)
                    nc.tensor.matmul(outp, ones_bf, pst["absb1"],
                                     start=False, stop=True)
                    pst["outp"] = outp
                elif g == 13:
                    osb = attn_sb.tile([1, 512], F32, tag="osb", bufs=2,
                                       name="osb")
                    nc.vector.tensor_copy(osb, pst["outp"])
                    nc.sync.dma_start(out=out_d[0:1, pisl], in_=osb)

            def attended(st, g, first, last):
                jh = g // (NG // 2)
                for t in range(2):
                    jloc = (2 * g + t) - jh * (NJC // 2)
                    for c2 in range(2):
                        nc.tensor.matmul(
                            st["attp"][c2],
                            vT_sb[jh][:, bass.ds(jloc * C + c2 * 128, 128)],
                            st["est"][g % 8][:, t, :],
                            start=(first and t == 0),
                            stop=(last and t == 1))

            for mi in range(NBLK + 1):
                cur = mi if mi < NBLK else None
                if cur is not None:
                    isl = bass.ts(cur, 512)
                    st = {
                        "attp": [ps_att.tile([128, 512], F32, tag="attp",
                                             bufs=3, name=f"attp{c2}")
                                 for c2 in range(2)],
                        "racc": [attn_sb.tile([128, 512], BF16, tag="racc",
                                              bufs=4, name=f"racc{t}")
                                 for t in range(2)],
                        "est": [None] * 8,
                    }
                    state[mi] = st
                for g in range(16 if cur is not None else 14):
                    if cur is not None:
                        jh = g // (NG // 2)
                        jloc = [(2 * g + t) - jh * (NJC // 2)
                                for t in range(2)]
                        stp = ps_st.tile([128, 2, 512], F32, tag="stp",
                                         bufs=2, name="stp")
                        for t in range(2):
                            nc.tensor.matmul(
                                stp[:, t, :],
                                k4_sb[jh][32 * t:32 * (t + 1),
                                          bass.ts(jloc[t], 128)],
                                q4_sb[32 * t:32 * (t + 1), isl],
                                start=True, stop=True,
                                tile_position=(32 * t, 0))
                        est = attn_sb.tile([128, 2, 512], BF16,
                                           tag="est", bufs=8, name="est")
                        st["est"][g % 8] = est
                        nc.scalar.activation(
                            est.rearrange("p a n -> p (a n)"),
                            stp.rearrange("p a n -> p (a n)"), AF.Exp)
                        # attended runs one group behind the scores
                        if g > 0:
                            attended(st, g - 1, first=(g == 1), last=False)
                        # DVE r-accumulation (bf16, 2 interleaved accums)
                        for t in range(2):
                            if g == 0:
                                nc.vector.tensor_copy(st["racc"][t],
                                                      est[:, t, :])
                            else:
                                nc.vector.tensor_add(st["racc"][t],
                                                     st["racc"][t],
                                                     est[:, t, :])
                    if mi > 0:
                        tail_ops(mi - 1, g, state[mi - 1])
                if cur is not None:
                    attended(st, 15, first=False, last=True)


_NC_CACHE = {}


def _get_nc():
    if "nc" not in _NC_CACHE:
        nc = bacc.Bacc("TRN2", debug=False, enable_asserts=False,
                       target_bir_lowering=False, enable_partition_id=False)
        with tile.TileContext(nc) as tc:
            build_program(nc, tc)
        nc.compile()
        _NC_CACHE["nc"] = nc
    return _NC_CACHE["nc"]


def host_inputs(x1, x2, Wq, bq, Wk, bk, Wv, bv, Wc, bc):
    """Build the 8 per-core input maps (host-side sharding/layout only)."""
    f = np.float32
    x1 = np.asarray(x1, f); x2 = np.asarray(x2, f)
    Wq = np.asarray(Wq, f); bq = np.asarray(bq, f)
    Wk = np.asarray(Wk, f)
    Wv = np.asarray(Wv, f); bv = np.asarray(bv, f)
    Wc = np.asarray(Wc, f); bc = np.asarray(bc, f)

    # 4x row-replicated q/k projection weights -> q4/k4 [128, n] layouts
    Wq4 = np.tile(Wq, (4, 1))            # [128, 256]
    Wk4 = np.tile(Wk, (4, 1))
    wqt = np.ascontiguousarray(Wq4.T.reshape(2, 128, 128))
    wkt = np.ascontiguousarray(Wk4.T.reshape(2, 128, 128))
    bq4 = np.tile(bq, 4).reshape(128, 1).copy()
    wvt = Wv.T.reshape(2, 128, C).astype(ml_dtypes.bfloat16)
    WcT = np.ascontiguousarray(Wc.T)     # [512, 256]
    wctx = WcT[:C].reshape(2, 128, C).astype(ml_dtypes.bfloat16)
    wcta = WcT[C:].reshape(2, 128, C).astype(ml_dtypes.bfloat16)
    bce = (bc + Wc[:, C:] @ bv).reshape(2, 128).T.copy()   # [128, 2]

    in_maps = []
    bf = ml_dtypes.bfloat16
    for core in range(NCORES):
        b, br = divmod(core, 2)
        x1f = np.ascontiguousarray(x1[b].reshape(C, N).reshape(2, 128, N))
        x2f = np.ascontiguousarray(x2[b].reshape(C, N).reshape(2, 128, N))
        in_maps.append({
            "xqk": x1f if br == 0 else x2f,
            "xv": (x2f if br == 0 else x1f).astype(bf),
            "xcr": x1f.astype(bf),
            "wqt": wqt, "wkt": wkt, "wvt": wvt,
            "wctx": wctx, "wcta": wcta,
            "bq": bq4, "bce": bce,
        })
    return in_maps


def assemble(results):
    """results: list of 8 dicts with 'out' [1, N] -> (out1, out2) full."""
    outs = []
    for br in range(2):
        full = np.empty((B, 1, HH, WW), np.float32)
        for b in range(B):
            full[b, 0] = results[2 * b + br]["out"][0].reshape(HH, WW)
        outs.append(full)
    return outs[0], outs[1]


def kernel(x1, x2, Wq, bq, Wk, bk, Wv, bv, Wc, bc):
    in_maps = host_inputs(x1, x2, Wq, bq, Wk, bk, Wv, bv, Wc, bc)
    nc = _get_nc()
    res = run_bass_kernel_spmd(nc, in_maps, core_ids=list(range(NCORES)))
    return assemble(res.results)
